# revision 37
# baseline (speedup 1.0000x reference)
"""EuclidConv + training-mode BatchNorm on 8 Trainium2 NeuronCores.

Math (reference): out = BN(2*conv(x,w) + conv(x^2, ones3x3) + ||w_f||^2),
BN over global-batch stats. Channel-constant terms are cancelled by BN's
mean subtraction, so ||w||^2 is never computed. Further, the x^2 channel
sums are centered by a flat -128 (the ACT cast's bias): after the 3x3 box
filter this becomes a uniform -1152 shift of every output pixel -- also
BN-cancelled -- which removes the count-map re-add matmuls entirely and
zero-centers s, making an fp16 s buffer precise enough.

Sharding: HYBRID. core c -> (chgrp = c//4, bgrp = c%4): 128 of 256 output
channels x 8 of 32 images -> full-width M=128 matmuls. BN statistics are
reduced across the 4 bgrps sharing a channel group via one tiny 4-rank
AllGather of [128,2] (sum, sumsq) + local fold.

Per image m (padded 30-row x 32-pitch grid, fp16; 32-pitch keeps every
box-filter tap 4-byte aligned so DVE runs in 2x pair mode):
  u_m = x_m^2                          (even m DVE 2x, odd m ACT)
  r4 psum = ones128.T @ u_m            (channel sums of x^2; 2 MMs)
  rc = ACT(r4, Identity, bias=-128)    (cast+center in one ACT op)
  vv = rc[0]+rc[+32]+rc[+64]           (vertical 3-tap, DVE 2x)
  te = vv[0]+vv[+2]                    (DVE 2x)
  tf = te+vv[+1]                       (GpSimd; odd offset is 1x anywhere)
Conv accumulation group per (img, yt-half) [128,392] psum (one bank),
group-major (9 k-offsets per group then next group) so each group's bank
drains ~1.5us before the chunk ends -> the next chunk's matmuls never
wait on a drain. LDWEIGHTS is pipelined by the PE, so no k-major sharing
is needed. Drain fuses the t1 add: DVE scalar_tensor_tensor
s = tf + psum -> s_sb (fp16) with accum S; squares for Q on DVE
(tensor_tensor_reduce, 2x) or ACT, balanced.
Stats: accum slots -> [128,2]; 4-rank AllGather via HBM bounce; fold;
A = gamma*rsqrt(var+eps), B = beta - mean*A; out = s*A+B (DVE/ACT) ->
dual-queue DMA out.

A dummy AllGather triggered at kernel start absorbs the ~25-45us NRT
first-collective entry barrier under compute. A memset-fed matmul warmup
flips the PE HAM clock gate to full rate during the input-DMA window.

Host-side prep is layout/sharding only: pad+transpose+cast of x, weight
transpose/scale, gamma/beta/eps packing.
"""
import json

import numpy as np

import concourse.bass as bass
import concourse.mybir as mybir
import concourse.tile as tile
from concourse.ap import AP
from concourse.bass_utils import run_bass_kernel_spmd
from concourse.vector_clock import ScopedClock, VectorClock

F16 = mybir.dt.float16
F32 = mybir.dt.float32
F8 = mybir.dt.float8e4
W8S = 16.0  # host pre-scale on (2w) so fp8 w stays clear of subnormals

N_CORES = 8
NIMG_L = 8  # images per core
ROWS = 30  # padded rows stored per image
PITCH = 32  # row pitch (4B-aligned taps -> DVE 2x mode)
NPIXL = ROWS * PITCH  # 960
NV = 28 * 28
NHW = 32 * NV  # global batch pixels per channel
EPS = 1e-5
CC_GROUPS = [[0, 1, 2, 3], [4, 5, 6, 7]]
N_WARM = 30

_split_ctr = [0]


def _split_waits_json(bir: bytes, max_waits: int = 1) -> bytes:
    """This container's walrus rejects instructions with >1 sync wait.
    Hoist excess waits onto EventSemaphore instructions inserted before the
    offender on the same engine stream."""
    m = json.loads(bir)
    for f in m["functions"]:
        for bb in f["blocks"]:
            newinsts = []
            for ins in bb["instructions"]:
                si = ins.get("sync_info")
                if si:
                    waits = si.get("on_wait") or []
                    if len(waits) > max_waits:
                        extra, keep = waits[:-max_waits], waits[-max_waits:]
                        for w_ in extra:
                            _split_ctr[0] += 1
                            newinsts.append(
                                {
                                    "debug": ins.get("debug", 0),
                                    "engine": ins["engine"],
                                    "ins": [],
                                    "outs": [],
                                    "name": f"antsplitw-{_split_ctr[0]}",
                                    "opcode": "EventSemaphore",
                                    "sync_info": {"on_update": [], "on_wait": [w_]},
                                }
                            )
                        si["on_wait"] = keep
                newinsts.append(ins)
            bb["instructions"] = newinsts
    return json.dumps(m).encode()


class _PatchedBass(bass.Bass):
    def to_json_bytes(self):
        return _split_waits_json(super().to_json_bytes())


class _SplitDrainTileContext(tile.TileContext):
    """Split the tile-exit drain's waits into single-wait drains (same
    walrus limitation as above)."""

    def _drain_and_barrier(self, tick_clock, wait_clock):
        g = tick_clock.global_clock
        n = len(g)
        for i in range(n):
            if g[i] > 0:
                vec = [0] * n
                vec[i] = g[i]
                d = self.nc.sync.drain()
                wait_clock.add_sem_waits(d.ins, ScopedClock({None: VectorClock(vec)}))
        self.nc.sync.drain()
        self.nc.all_engine_barrier()
        assert self.sems is not None
        popped = self.nc._tile_sem_poison_stack.pop()
        assert popped is self._sem_poison
        self.nc.clear_and_free_semaphores(list(self.sems.allocated().values()))
        self.nc.all_engine_barrier()


def _build_nc():
    nc = _PatchedBass(num_devices=N_CORES)
    xh = nc.dram_tensor("xh", [128, NIMG_L * NPIXL], F16, kind="ExternalInput")
    wt = nc.dram_tensor("wt", [128, 9 * 128], F16, kind="ExternalInput")
    cst32d = nc.dram_tensor("cst32", [128, 3], F32, kind="ExternalInput")
    y = nc.dram_tensor("y", [NIMG_L, 128, 28, 28], F16, kind="ExternalOutput")

    with _SplitDrainTileContext(nc) as tc:
        with (
            tc.tile_pool(name="const", bufs=1) as cpool,
            tc.tile_pool(name="xpool", bufs=1) as xpool,
            tc.tile_pool(name="upool", bufs=4) as upool,
            tc.tile_pool(name="boxp", bufs=3) as boxp,
            tc.tile_pool(name="tfp", bufs=4) as tfp,
            tc.tile_pool(name="spool", bufs=1) as spool,
            tc.tile_pool(name="opool", bufs=8) as opool,
            tc.tile_pool(name="psr", bufs=2, space="PSUM") as psr,
            tc.tile_pool(name="psc", bufs=4, space="PSUM") as psc,
            tc.tile_pool(name="dram", bufs=1, space="DRAM") as dram,
        ):
            # ---- dummy collective, triggered first: absorbs the NRT entry
            # barrier + first-collective ncfw setup under compute ----
            dcin = dram.tile([2, 2], F32, name="dcin")
            dcout = dram.tile([2 * 4, 2], F32, name="dcout")
            nc.gpsimd.collective_compute(
                "AllGather",
                mybir.AluOpType.bypass,
                replica_groups=CC_GROUPS,
                ins=[dcin[:].opt()],
                outs=[dcout[:].opt()],
            )

            # ---- input DMAs, criticality-ordered ----
            xall = xpool.tile([128, NIMG_L * NPIXL], F16, name="xall")

            def ximg(m):
                nc.sync.dma_start(
                    xall[:, m * NPIXL : (m + 1) * NPIXL],
                    xh[:, m * NPIXL : (m + 1) * NPIXL],
                )

            ximg(0)
            ximg(1)
            wtile = cpool.tile([128, 9 * 128], F16, name="wtile")
            nc.sync.dma_start(wtile[:], wt[:])
            for _m in range(2, NIMG_L):
                ximg(_m)
            c32 = cpool.tile([128, 3], F32, name="c32")
            nc.scalar.dma_start(c32[:], cst32d[:])

            ones128 = cpool.tile([128, 128], F16, name="ones128")
            nc.vector.memset(ones128[:], 1.0)
            cm128 = cpool.tile([128, 1], F32, name="cm128")
            nc.vector.memset(cm128[:], -384.0)

            x3 = xall[:].rearrange("p (n a b) -> p n a b", a=ROWS, b=PITCH)

            s_sb = spool.tile([128, NIMG_L * NV], F16, name="s_sb")
            sums16 = spool.tile([128, 2 * NIMG_L], F32, name="sums16")
            sumsq = spool.tile([128, NIMG_L + 1], F32, name="sumsq")

            # ---- PE warmup: flip HAM to full rate during the input-DMA
            # window; memset-fed so it has no DMA dependency ----
            wsrc = cpool.tile([128, 128], F16, name="wsrc")
            nc.vector.memset(wsrc[:], 0.25)
            warm = psr.tile([128, 1024], F32, name="warm", tag="r4")
            for _ in range(N_WARM):
                nc.tensor.matmul(
                    warm[:, 0:128], wsrc[:], wsrc[:], start=True, stop=True,
                    skip_group_check=True,
                )

            # ---- ACT spline-table preload (first activation pays ~1.3us) ----
            tscr = spool.tile([128, 8], F32, name="tscr")
            nc.vector.memset(tscr[:, 0:4], 1.0)
            nc.scalar.activation(
                tscr[:, 4:8], tscr[:, 0:4], mybir.ActivationFunctionType.Square
            )

            uts = [None] * NIMG_L
            tfs = [None] * NIMG_L
            rcs = [None] * NIMG_L

            def emit_u(m):
                ut = upool.tile([128, NPIXL], F16, name=f"u{m}", tag="u")
                xs = xall[:, m * NPIXL : (m + 1) * NPIXL]
                nc.scalar.activation(
                    ut[:], xs, mybir.ActivationFunctionType.Square
                )
                uts[m] = ut

            def emit_h(m):
                """Horizontal 3-tap of the x^2 channel sums, all on PE:
                3 column-shifted all-ones matmuls accumulate into one psum
                group per bank; ACT casts psum -> fp16 with a flat -384
                bias (the centering; BN cancels the resulting uniform
                -1152 after the vertical pass)."""
                r4 = psr.tile([128, 1024], F32, name=f"r4_{m}", tag="r4")
                for lo, hi in ((0, 512), (512, 958)):
                    for s in range(3):
                        nc.tensor.matmul(
                            r4[:, lo:hi],
                            ones128[:],
                            uts[m][:, lo + s : hi + s],
                            start=(s == 0),
                            stop=(s == 2),
                            skip_group_check=True,
                        )
                rc = boxp.tile([128, 958], F16, name=f"rc{m}", tag="rc")
                nc.scalar.activation(
                    rc[:, 0:958],
                    r4[:, 0:958],
                    mybir.ActivationFunctionType.Identity,
                    bias=cm128[:, 0:1],
                )
                rcs[m] = rc

            def emit_vert(m):
                """Vertical 3-tap on DVE finishes the 3x3 box."""
                rc = rcs[m]
                vv = boxp.tile([128, 894], F16, name=f"vv{m}", tag="vv")
                tf = tfp.tile([128, 894], F16, name=f"tf{m}", tag="tf")
                nc.vector.tensor_add(vv[:, 0:894], rc[:, 0:894], rc[:, 32:926])
                nc.vector.tensor_add(tf[:, 0:894], vv[:, 0:894], rc[:, 64:958])
                tfs[m] = tf

            def stt_drain(m, yt, ps):
                off = m * NV + yt * 392
                t13v = tfs[m]
                sdst = AP(
                    s_sb.tensor,
                    s_sb.offset + off,
                    [[NIMG_L * NV, 128], [28, 14], [1, 28]],
                )
                tfv = AP(
                    t13v.tensor,
                    t13v.offset + yt * 448,
                    [[894, 128], [32, 14], [1, 28]],
                )
                psv = AP(ps.tensor, ps.offset, [[512, 128], [28, 14], [1, 28]])
                nc.vector.scalar_tensor_tensor(
                    sdst,
                    psv,
                    1.0,
                    tfv,
                    op0=mybir.AluOpType.mult,
                    op1=mybir.AluOpType.add,
                    accum_out=sums16[:, 2 * m + yt : 2 * m + yt + 1],
                )

            def emit_ssq(m, lo=0, hi=NV, slot=None):
                blk = m * NV
                slot = m if slot is None else slot
                sq = opool.tile([128, NV], F16, name=f"sq{m}_{lo}", tag="sq")
                if m == NIMG_L - 1:
                    # last image: DVE, so the final sumsq isn't queued
                    # behind ACT's rc casts at the stats critical path
                    nc.vector.scalar_tensor_tensor(
                        sq[:, lo:hi],
                        s_sb[:, blk + lo : blk + hi],
                        1.0,
                        s_sb[:, blk + lo : blk + hi],
                        op0=mybir.AluOpType.mult,
                        op1=mybir.AluOpType.mult,
                        accum_out=sumsq[:, slot : slot + 1],
                    )
                else:
                    nc.scalar.activation(
                        sq[:, lo:hi],
                        s_sb[:, blk + lo : blk + hi],
                        mybir.ActivationFunctionType.Square,
                        accum_out=sumsq[:, slot : slot + 1],
                    )

            def conv_chunk(b):
                """Conv groups for images 2b, 2b+1, group-major. Mid-chunk
                hooks prep the next chunk's u/r4/rc/box so the PE, DVE, ACT
                and GP queues all stay fed across chunk boundaries."""
                ms = (2 * b, 2 * b + 1)
                groups = [(m, yt) for m in ms for yt in range(2)]
                nxt = (2 * b + 2, 2 * b + 3) if b < 3 else ()
                for gi, (m, yt) in enumerate(groups):
                    ps = psc.tile([128, 512], F32, name=f"ps{m}_{yt}", tag="ps")
                    y0 = yt * 14
                    for k in range(9):
                        dy, dx = divmod(k, 3)
                        nc.tensor.matmul(
                            ps[:, 0:392],
                            wtile[:, k * 128 : (k + 1) * 128],
                            x3[:, m, y0 + dy : y0 + dy + 14, dx : dx + 28],
                            start=(k == 0),
                            stop=(k == 8),
                            skip_group_check=True,
                        )
                    stt_drain(m, yt, ps)
                    if gi == 0:
                        for mn in nxt:
                            emit_u(mn)
                    elif gi == 1:
                        emit_ssq(ms[0])
                        for mn in nxt:
                            emit_h(mn)
                    elif gi == 2:
                        for mn in nxt:
                            emit_vert(mn)
                        if b == 3:
                            emit_ssq(7, 0, 392, slot=7)
                if b == 3:
                    emit_ssq(7, 392, NV, slot=8)
                else:
                    emit_ssq(ms[1])

            # prologue: first two images' chains feed conv_chunk(0)
            emit_u(0)
            emit_u(1)
            emit_h(0)
            emit_h(1)
            emit_vert(0)
            emit_vert(1)
            conv_chunk(0)
            conv_chunk(1)
            # ---- deliberate ~2us PE-slack window: the ncfw entry-barrier
            # handshake only advances when the PE queue has slack; without
            # this it completes just before conv-end and the stats
            # collective serializes behind the dummy ~20us late. A DVE
            # memset gates one throwaway matmul so the PE idles here. ----
            bub = cpool.tile([128, 1536], F16, name="bub")
            nc.vector.memset(bub[:], 0.25)
            bwm = psr.tile([128, 1024], F32, name="bwm", tag="r4")
            nc.tensor.matmul(
                bwm[:, 0:128], bub[:, 0:128], bub[:, 128:256],
                start=True, stop=True, skip_group_check=True,
            )
            conv_chunk(2)
            conv_chunk(3)

            # ---- stats: local fold -> 4-rank AllGather -> global fold ----
            st2 = spool.tile([128, 2], F32, name="st2")
            nc.vector.tensor_reduce(
                out=st2[:, 0:1], in_=sums16[:], op=mybir.AluOpType.add,
                axis=mybir.AxisListType.X,
            )
            nc.vector.tensor_reduce(
                out=st2[:, 1:2], in_=sumsq[:], op=mybir.AluOpType.add,
                axis=mybir.AxisListType.X,
            )
            cin = dram.tile([128, 2], F32, name="cin")
            cout = dram.tile([128 * 4, 2], F32, name="cout")
            nc.scalar.dma_start(cin[:], st2[:])
            nc.gpsimd.collective_compute(
                "AllGather",
                mybir.AluOpType.bypass,
                replica_groups=CC_GROUPS,
                ins=[cin[:].opt()],
                outs=[cout[:].opt()],
            )
            g = spool.tile([128, 8], F32, name="g")
            nc.sync.dma_start(
                g[:], AP(cout.tensor, cout.offset, [[2, 128], [256, 4], [1, 2]])
            )
            gs = spool.tile([128, 2], F32, name="gs")
            nc.vector.tensor_add(gs[:], g[:, 0:2], g[:, 2:4])
            nc.vector.tensor_add(gs[:], gs[:], g[:, 4:6])
            nc.vector.tensor_add(gs[:], gs[:], g[:, 6:8])

            ab = spool.tile([128, 8], F32, name="ab")
            mean = ab[:, 0:1]
            qn = ab[:, 1:2]
            nc.vector.tensor_scalar_mul(ab[:, 0:2], gs[:, 0:2], 1.0 / NHW)
            var = ab[:, 2:3]
            nc.vector.scalar_tensor_tensor(
                var, mean, 1.0, mean, op0=mybir.AluOpType.mult,
                op1=mybir.AluOpType.mult,
            )
            nc.vector.tensor_sub(var, qn, var)
            sd = ab[:, 3:4]
            nc.scalar.activation(
                sd, var, mybir.ActivationFunctionType.Sqrt, bias=c32[:, 2:3]
            )
            abv = spool.tile([128, 2], F32, name="abv")
            A = abv[:, 0:1]
            B = abv[:, 1:2]
            nc.vector.reciprocal(A, sd)
            nc.vector.tensor_mul(A, A, c32[:, 0:1])
            nc.vector.scalar_tensor_tensor(
                B, mean, 1.0, A, op0=mybir.AluOpType.mult, op1=mybir.AluOpType.mult
            )
            nc.vector.tensor_sub(B, c32[:, 1:2], B)

            # ---- normalize + store (engine balance: ACT is slower/op) ----
            for m in range(NIMG_L):
                blk = m * NV
                o = opool.tile([128, NV], F16, name=f"o{m}", tag="o")
                if m in (1, 5):
                    nc.scalar.activation(
                        o[:],
                        s_sb[:, blk : blk + NV],
                        mybir.ActivationFunctionType.Identity,
                        bias=B,
                        scale=A,
                    )
                else:
                    nc.vector.tensor_scalar(
                        o[:],
                        s_sb[:, blk : blk + NV],
                        A,
                        B,
                        op0=mybir.AluOpType.mult,
                        op1=mybir.AluOpType.add,
                    )
                dst = AP(y.ap().tensor, m * 128 * NV, [[NV, 128], [1, NV]])
                eng = nc.sync if m % 2 == 0 else nc.scalar
                eng.dma_start(dst, o[:])
    return nc


def _prep_inputs(x, w, gamma, beta):
    import ml_dtypes

    x = np.asarray(x, np.float32)
    w = np.asarray(w, np.float32)
    gamma = np.asarray(gamma, np.float32)
    beta = np.asarray(beta, np.float32)

    xp = np.zeros((32, 128, ROWS, PITCH), np.float32)
    xp[:, :, 1:29, 1:29] = x

    maps = []
    for core in range(N_CORES):
        cg, bg = core // 4, core % 4
        xs = xp[bg * NIMG_L : (bg + 1) * NIMG_L]
        xhc = np.ascontiguousarray(xs.transpose(1, 0, 2, 3)).reshape(
            128, NIMG_L * NPIXL
        )
        wc = (2.0 * w[cg * 128 : (cg + 1) * 128]).reshape(128, 128, 9)
        wtc = np.ascontiguousarray(wc.transpose(1, 2, 0)).reshape(128, 9 * 128)
        cst32 = np.zeros((128, 3), np.float32)
        cst32[:, 0] = gamma[cg * 128 : (cg + 1) * 128]
        cst32[:, 1] = beta[cg * 128 : (cg + 1) * 128]
        cst32[:, 2] = EPS
        maps.append(
            {
                "xh": xhc.astype(np.float16),
                "wt": wtc.astype(np.float16),
                "cst32": cst32,
            }
        )
    return maps


_NC_CACHE = []


def _assemble(results):
    out = np.empty((32, 256, 28, 28), np.float32)
    for core in range(N_CORES):
        cg, bg = core // 4, core % 4
        out[bg * NIMG_L : (bg + 1) * NIMG_L, cg * 128 : (cg + 1) * 128] = (
            results[core]["y"].astype(np.float32)
        )
    return out


def kernel(x, w, gamma, beta):
    if not _NC_CACHE:
        _NC_CACHE.append(_build_nc())
    nc = _NC_CACHE[0]
    maps = _prep_inputs(x, w, gamma, beta)
    res = run_bass_kernel_spmd(nc, maps, core_ids=list(range(N_CORES)))
    return _assemble(res.results)


# revision 42
# speedup vs baseline: 1.0591x; 1.0591x over previous
"""EuclidConv + training-mode BatchNorm on 8 Trainium2 NeuronCores.

Math (reference): out = BN(2*conv(x,w) + conv(x^2, ones3x3) + ||w_f||^2),
BN over global-batch stats. Channel-constant terms are cancelled by BN's
mean subtraction, so ||w||^2 is never computed. Further, the x^2 channel
sums are centered by a flat -128 (the ACT cast's bias): after the 3x3 box
filter this becomes a uniform -1152 shift of every output pixel -- also
BN-cancelled -- which removes the count-map re-add matmuls entirely and
zero-centers s, making an fp16 s buffer precise enough.

Sharding: HYBRID. core c -> (chgrp = c//4, bgrp = c%4): 128 of 256 output
channels x 8 of 32 images -> full-width M=128 matmuls. BN statistics are
reduced across the 4 bgrps sharing a channel group via one tiny 4-rank
AllGather of [128,2] (sum, sumsq) + local fold.

Per image m (padded 30-row x 32-pitch grid, fp16; 32-pitch keeps every
box-filter tap 4-byte aligned so DVE runs in 2x pair mode):
  u_m = x_m^2                          (even m DVE 2x, odd m ACT)
  r4 psum = ones128.T @ u_m            (channel sums of x^2; 2 MMs)
  rc = ACT(r4, Identity, bias=-128)    (cast+center in one ACT op)
  vv = rc[0]+rc[+32]+rc[+64]           (vertical 3-tap, DVE 2x)
  te = vv[0]+vv[+2]                    (DVE 2x)
  tf = te+vv[+1]                       (GpSimd; odd offset is 1x anywhere)
Conv accumulation group per (img, yt-half) [128,392] psum (one bank),
group-major (9 k-offsets per group then next group) so each group's bank
drains ~1.5us before the chunk ends -> the next chunk's matmuls never
wait on a drain. LDWEIGHTS is pipelined by the PE, so no k-major sharing
is needed. Drain fuses the t1 add: DVE scalar_tensor_tensor
s = tf + psum -> s_sb (fp16) with accum S; squares for Q on DVE
(tensor_tensor_reduce, 2x) or ACT, balanced.
Stats: accum slots -> [128,2]; 4-rank AllGather via HBM bounce; fold;
A = gamma*rsqrt(var+eps), B = beta - mean*A; out = s*A+B (DVE/ACT) ->
dual-queue DMA out.

A dummy AllGather triggered at kernel start absorbs the ~25-45us NRT
first-collective entry barrier under compute. A memset-fed matmul warmup
flips the PE HAM clock gate to full rate during the input-DMA window.

Host-side prep is layout/sharding only: pad+transpose+cast of x, weight
transpose/scale, gamma/beta/eps packing.
"""
import json

import numpy as np

import concourse.bass as bass
import concourse.mybir as mybir
import concourse.tile as tile
from concourse.ap import AP
from concourse.bass_utils import run_bass_kernel_spmd
from concourse.vector_clock import ScopedClock, VectorClock

F16 = mybir.dt.float16
F32 = mybir.dt.float32
F8 = mybir.dt.float8e4
W8S = 16.0  # host pre-scale on (2w) so fp8 w stays clear of subnormals

N_CORES = 8
NIMG_L = 8  # images per core
ROWS = 30  # padded rows stored per image
PITCH = 32  # row pitch (4B-aligned taps -> DVE 2x mode)
NPIXL = ROWS * PITCH  # 960
NV = 28 * 28
NHW = 32 * NV  # global batch pixels per channel
EPS = 1e-5
CC_GROUPS = [[0, 1, 2, 3], [4, 5, 6, 7]]
N_WARM = 30

_split_ctr = [0]


def _split_waits_json(bir: bytes, max_waits: int = 1) -> bytes:
    """This container's walrus rejects instructions with >1 sync wait.
    Hoist excess waits onto EventSemaphore instructions inserted before the
    offender on the same engine stream."""
    m = json.loads(bir)
    for f in m["functions"]:
        for bb in f["blocks"]:
            newinsts = []
            for ins in bb["instructions"]:
                si = ins.get("sync_info")
                if si:
                    waits = si.get("on_wait") or []
                    if len(waits) > max_waits:
                        extra, keep = waits[:-max_waits], waits[-max_waits:]
                        for w_ in extra:
                            _split_ctr[0] += 1
                            newinsts.append(
                                {
                                    "debug": ins.get("debug", 0),
                                    "engine": ins["engine"],
                                    "ins": [],
                                    "outs": [],
                                    "name": f"antsplitw-{_split_ctr[0]}",
                                    "opcode": "EventSemaphore",
                                    "sync_info": {"on_update": [], "on_wait": [w_]},
                                }
                            )
                        si["on_wait"] = keep
                newinsts.append(ins)
            bb["instructions"] = newinsts
    return json.dumps(m).encode()


class _PatchedBass(bass.Bass):
    def to_json_bytes(self):
        return _split_waits_json(super().to_json_bytes())


class _SplitDrainTileContext(tile.TileContext):
    """Split the tile-exit drain's waits into single-wait drains (same
    walrus limitation as above)."""

    def _drain_and_barrier(self, tick_clock, wait_clock):
        g = tick_clock.global_clock
        n = len(g)
        for i in range(n):
            if g[i] > 0:
                vec = [0] * n
                vec[i] = g[i]
                d = self.nc.sync.drain()
                wait_clock.add_sem_waits(d.ins, ScopedClock({None: VectorClock(vec)}))
        self.nc.sync.drain()
        self.nc.all_engine_barrier()
        assert self.sems is not None
        popped = self.nc._tile_sem_poison_stack.pop()
        assert popped is self._sem_poison
        self.nc.clear_and_free_semaphores(list(self.sems.allocated().values()))
        self.nc.all_engine_barrier()


def _build_nc():
    nc = _PatchedBass(num_devices=N_CORES)
    xh = nc.dram_tensor("xh", [128, NIMG_L * NPIXL], F16, kind="ExternalInput")
    wt = nc.dram_tensor("wt", [128, 9 * 128], F16, kind="ExternalInput")
    cst32d = nc.dram_tensor("cst32", [128, 3], F32, kind="ExternalInput")
    y = nc.dram_tensor("y", [NIMG_L, 128, 28, 28], F16, kind="ExternalOutput")

    with _SplitDrainTileContext(nc) as tc:
        with (
            tc.tile_pool(name="const", bufs=1) as cpool,
            tc.tile_pool(name="xpool", bufs=1) as xpool,
            tc.tile_pool(name="upool", bufs=4) as upool,
            tc.tile_pool(name="boxp", bufs=3) as boxp,
            tc.tile_pool(name="tfp", bufs=4) as tfp,
            tc.tile_pool(name="spool", bufs=1) as spool,
            tc.tile_pool(name="opool", bufs=8) as opool,
            tc.tile_pool(name="psr", bufs=2, space="PSUM") as psr,
            tc.tile_pool(name="psc", bufs=4, space="PSUM") as psc,
            tc.tile_pool(name="dram", bufs=1, space="DRAM") as dram,
        ):
            # ---- dummy collective, triggered first: absorbs the NRT entry
            # barrier + first-collective ncfw setup under compute ----
            dcin = dram.tile([2, 2], F32, name="dcin")
            dcout = dram.tile([2 * 4, 2], F32, name="dcout")
            nc.gpsimd.collective_compute(
                "AllGather",
                mybir.AluOpType.bypass,
                replica_groups=CC_GROUPS,
                ins=[dcin[:].opt()],
                outs=[dcout[:].opt()],
            )

            # ---- input DMAs, criticality-ordered ----
            xall = xpool.tile([128, NIMG_L * NPIXL], F16, name="xall")

            def ximg(m):
                nc.sync.dma_start(
                    xall[:, m * NPIXL : (m + 1) * NPIXL],
                    xh[:, m * NPIXL : (m + 1) * NPIXL],
                )

            ximg(0)
            ximg(1)
            wtile = cpool.tile([128, 9 * 128], F16, name="wtile")
            nc.sync.dma_start(wtile[:], wt[:])
            for _m in range(2, NIMG_L):
                ximg(_m)
            c32 = cpool.tile([128, 3], F32, name="c32")
            nc.scalar.dma_start(c32[:], cst32d[:])

            ones128 = cpool.tile([128, 128], F16, name="ones128")
            nc.vector.memset(ones128[:], 1.0)
            cm128 = cpool.tile([128, 1], F32, name="cm128")
            nc.vector.memset(cm128[:], -384.0)

            x3 = xall[:].rearrange("p (n a b) -> p n a b", a=ROWS, b=PITCH)

            s_sb = spool.tile([128, NIMG_L * NV], F16, name="s_sb")
            sums16 = spool.tile([128, 2 * NIMG_L], F32, name="sums16")
            sumsq = spool.tile([128, NIMG_L + 1], F32, name="sumsq")

            # ---- PE warmup: flip HAM to full rate during the input-DMA
            # window; memset-fed so it has no DMA dependency ----
            wsrc = cpool.tile([128, 128], F16, name="wsrc")
            nc.vector.memset(wsrc[:], 0.25)
            warm = psr.tile([128, 1024], F32, name="warm", tag="r4")
            for _ in range(N_WARM):
                nc.tensor.matmul(
                    warm[:, 0:128], wsrc[:], wsrc[:], start=True, stop=True,
                    skip_group_check=True,
                )

            # ---- ACT spline-table preload (first activation pays ~1.3us) ----
            tscr = spool.tile([128, 8], F32, name="tscr")
            nc.vector.memset(tscr[:, 0:4], 1.0)
            nc.scalar.activation(
                tscr[:, 4:8], tscr[:, 0:4], mybir.ActivationFunctionType.Square
            )

            uts = [None] * NIMG_L
            tfs = [None] * NIMG_L
            rcs = [None] * NIMG_L

            def emit_u(m):
                ut = upool.tile([128, NPIXL], F16, name=f"u{m}", tag="u")
                xs = xall[:, m * NPIXL : (m + 1) * NPIXL]
                nc.scalar.activation(
                    ut[:], xs, mybir.ActivationFunctionType.Square
                )
                uts[m] = ut

            def emit_h(m):
                """Horizontal 3-tap of the x^2 channel sums, all on PE:
                3 column-shifted all-ones matmuls accumulate into one psum
                group per bank; ACT casts psum -> fp16 with a flat -384
                bias (the centering; BN cancels the resulting uniform
                -1152 after the vertical pass)."""
                r4 = psr.tile([128, 1024], F32, name=f"r4_{m}", tag="r4")
                for lo, hi in ((0, 512), (512, 958)):
                    for s in range(3):
                        nc.tensor.matmul(
                            r4[:, lo:hi],
                            ones128[:],
                            uts[m][:, lo + s : hi + s],
                            start=(s == 0),
                            stop=(s == 2),
                            skip_group_check=True,
                        )
                rc = boxp.tile([128, 958], F16, name=f"rc{m}", tag="rc")
                nc.scalar.activation(
                    rc[:, 0:958],
                    r4[:, 0:958],
                    mybir.ActivationFunctionType.Identity,
                    bias=cm128[:, 0:1],
                )
                rcs[m] = rc

            def emit_vert(m):
                """Vertical 3-tap on DVE finishes the 3x3 box."""
                rc = rcs[m]
                vv = boxp.tile([128, 894], F16, name=f"vv{m}", tag="vv")
                tf = tfp.tile([128, 894], F16, name=f"tf{m}", tag="tf")
                nc.vector.tensor_add(vv[:, 0:894], rc[:, 0:894], rc[:, 32:926])
                nc.vector.tensor_add(tf[:, 0:894], vv[:, 0:894], rc[:, 64:958])
                tfs[m] = tf

            def stt_drain(m, yt, ps):
                off = m * NV + yt * 392
                t13v = tfs[m]
                sdst = AP(
                    s_sb.tensor,
                    s_sb.offset + off,
                    [[NIMG_L * NV, 128], [28, 14], [1, 28]],
                )
                tfv = AP(
                    t13v.tensor,
                    t13v.offset + yt * 448,
                    [[894, 128], [32, 14], [1, 28]],
                )
                psv = AP(ps.tensor, ps.offset, [[512, 128], [28, 14], [1, 28]])
                nc.vector.scalar_tensor_tensor(
                    sdst,
                    psv,
                    1.0,
                    tfv,
                    op0=mybir.AluOpType.mult,
                    op1=mybir.AluOpType.add,
                    accum_out=sums16[:, 2 * m + yt : 2 * m + yt + 1],
                )

            def emit_ssq(m, lo=0, hi=NV, slot=None):
                blk = m * NV
                slot = m if slot is None else slot
                sq = opool.tile([128, NV], F16, name=f"sq{m}_{lo}", tag="sq")
                if m == NIMG_L - 1:
                    # last image: DVE, so the final sumsq isn't queued
                    # behind ACT's rc casts at the stats critical path
                    nc.vector.scalar_tensor_tensor(
                        sq[:, lo:hi],
                        s_sb[:, blk + lo : blk + hi],
                        1.0,
                        s_sb[:, blk + lo : blk + hi],
                        op0=mybir.AluOpType.mult,
                        op1=mybir.AluOpType.mult,
                        accum_out=sumsq[:, slot : slot + 1],
                    )
                else:
                    nc.scalar.activation(
                        sq[:, lo:hi],
                        s_sb[:, blk + lo : blk + hi],
                        mybir.ActivationFunctionType.Square,
                        accum_out=sumsq[:, slot : slot + 1],
                    )

            pscr = spool.tile([128, 2], F32, name="pscr")

            def gp_poke(m):
                """Paced GpSimd wakeup. An empty (parked) GP queue starves
                the ncfw collective-service loop: the NRT entry barrier's
                handshake steps then only advance every ~20us and the stats
                AllGather queues up behind a late dummy. One tiny GP op per
                image, gated on that image's drain accumulators, wakes the
                engine every ~4us with idle gaps between -- the pattern
                under which the barrier completes early."""
                blk = m * NV
                nc.gpsimd.tensor_add(
                    pscr[:, 0:1],
                    s_sb[:, blk : blk + 1],
                    s_sb[:, blk + 392 : blk + 393],
                )

            def conv_chunk(b):
                """Conv groups for images 2b, 2b+1, group-major. Mid-chunk
                hooks prep the next chunk's u/r4/rc/box so the PE, DVE, ACT
                and GP queues all stay fed across chunk boundaries."""
                ms = (2 * b, 2 * b + 1)
                groups = [(m, yt) for m in ms for yt in range(2)]
                nxt = (2 * b + 2, 2 * b + 3) if b < 3 else ()
                for gi, (m, yt) in enumerate(groups):
                    ps = psc.tile([128, 512], F32, name=f"ps{m}_{yt}", tag="ps")
                    y0 = yt * 14
                    for k in range(9):
                        dy, dx = divmod(k, 3)
                        nc.tensor.matmul(
                            ps[:, 0:392],
                            wtile[:, k * 128 : (k + 1) * 128],
                            x3[:, m, y0 + dy : y0 + dy + 14, dx : dx + 28],
                            start=(k == 0),
                            stop=(k == 8),
                            skip_group_check=True,
                        )
                    stt_drain(m, yt, ps)
                    if gi == 0:
                        for mn in nxt:
                            emit_u(mn)
                    elif gi == 1:
                        gp_poke(ms[0])
                        emit_ssq(ms[0])
                        for mn in nxt:
                            emit_h(mn)
                    elif gi == 2:
                        for mn in nxt:
                            emit_vert(mn)
                        if b == 3:
                            emit_ssq(7, 0, 392, slot=7)
                if b == 3:
                    emit_ssq(7, 392, NV, slot=8)
                else:
                    emit_ssq(ms[1])
                    gp_poke(ms[1])

            # prologue: first two images' chains feed conv_chunk(0)
            emit_u(0)
            emit_u(1)
            emit_h(0)
            emit_h(1)
            emit_vert(0)
            emit_vert(1)
            for b in range(4):
                conv_chunk(b)

            # ---- stats: local fold -> 4-rank AllGather -> global fold ----
            st2 = spool.tile([128, 2], F32, name="st2")
            nc.vector.tensor_reduce(
                out=st2[:, 0:1], in_=sums16[:], op=mybir.AluOpType.add,
                axis=mybir.AxisListType.X,
            )
            nc.vector.tensor_reduce(
                out=st2[:, 1:2], in_=sumsq[:], op=mybir.AluOpType.add,
                axis=mybir.AxisListType.X,
            )
            cin = dram.tile([128, 2], F32, name="cin")
            cout = dram.tile([128 * 4, 2], F32, name="cout")
            nc.scalar.dma_start(cin[:], st2[:])
            nc.gpsimd.collective_compute(
                "AllGather",
                mybir.AluOpType.bypass,
                replica_groups=CC_GROUPS,
                ins=[cin[:].opt()],
                outs=[cout[:].opt()],
            )
            g = spool.tile([128, 8], F32, name="g")
            nc.sync.dma_start(
                g[:], AP(cout.tensor, cout.offset, [[2, 128], [256, 4], [1, 2]])
            )
            gs = spool.tile([128, 2], F32, name="gs")
            nc.vector.tensor_add(gs[:], g[:, 0:2], g[:, 2:4])
            nc.vector.tensor_add(gs[:], gs[:], g[:, 4:6])
            nc.vector.tensor_add(gs[:], gs[:], g[:, 6:8])

            ab = spool.tile([128, 8], F32, name="ab")
            mean = ab[:, 0:1]
            qn = ab[:, 1:2]
            nc.vector.tensor_scalar_mul(ab[:, 0:2], gs[:, 0:2], 1.0 / NHW)
            var = ab[:, 2:3]
            nc.vector.scalar_tensor_tensor(
                var, mean, 1.0, mean, op0=mybir.AluOpType.mult,
                op1=mybir.AluOpType.mult,
            )
            nc.vector.tensor_sub(var, qn, var)
            sd = ab[:, 3:4]
            nc.scalar.activation(
                sd, var, mybir.ActivationFunctionType.Sqrt, bias=c32[:, 2:3]
            )
            abv = spool.tile([128, 2], F32, name="abv")
            A = abv[:, 0:1]
            B = abv[:, 1:2]
            nc.vector.reciprocal(A, sd)
            nc.vector.tensor_mul(A, A, c32[:, 0:1])
            nc.vector.scalar_tensor_tensor(
                B, mean, 1.0, A, op0=mybir.AluOpType.mult, op1=mybir.AluOpType.mult
            )
            nc.vector.tensor_sub(B, c32[:, 1:2], B)

            # ---- normalize + store (engine balance: ACT is slower/op) ----
            for m in range(NIMG_L):
                blk = m * NV
                o = opool.tile([128, NV], F16, name=f"o{m}", tag="o")
                if m in (1, 5):
                    nc.scalar.activation(
                        o[:],
                        s_sb[:, blk : blk + NV],
                        mybir.ActivationFunctionType.Identity,
                        bias=B,
                        scale=A,
                    )
                else:
                    nc.vector.tensor_scalar(
                        o[:],
                        s_sb[:, blk : blk + NV],
                        A,
                        B,
                        op0=mybir.AluOpType.mult,
                        op1=mybir.AluOpType.add,
                    )
                dst = AP(y.ap().tensor, m * 128 * NV, [[NV, 128], [1, NV]])
                eng = nc.sync if m % 2 == 0 else nc.scalar
                eng.dma_start(dst, o[:])
    return nc


def _prep_inputs(x, w, gamma, beta):
    import ml_dtypes

    x = np.asarray(x, np.float32)
    w = np.asarray(w, np.float32)
    gamma = np.asarray(gamma, np.float32)
    beta = np.asarray(beta, np.float32)

    xp = np.zeros((32, 128, ROWS, PITCH), np.float32)
    xp[:, :, 1:29, 1:29] = x

    maps = []
    for core in range(N_CORES):
        cg, bg = core // 4, core % 4
        xs = xp[bg * NIMG_L : (bg + 1) * NIMG_L]
        xhc = np.ascontiguousarray(xs.transpose(1, 0, 2, 3)).reshape(
            128, NIMG_L * NPIXL
        )
        wc = (2.0 * w[cg * 128 : (cg + 1) * 128]).reshape(128, 128, 9)
        wtc = np.ascontiguousarray(wc.transpose(1, 2, 0)).reshape(128, 9 * 128)
        cst32 = np.zeros((128, 3), np.float32)
        cst32[:, 0] = gamma[cg * 128 : (cg + 1) * 128]
        cst32[:, 1] = beta[cg * 128 : (cg + 1) * 128]
        cst32[:, 2] = EPS
        maps.append(
            {
                "xh": xhc.astype(np.float16),
                "wt": wtc.astype(np.float16),
                "cst32": cst32,
            }
        )
    return maps


_NC_CACHE = []


def _assemble(results):
    out = np.empty((32, 256, 28, 28), np.float32)
    for core in range(N_CORES):
        cg, bg = core // 4, core % 4
        out[bg * NIMG_L : (bg + 1) * NIMG_L, cg * 128 : (cg + 1) * 128] = (
            results[core]["y"].astype(np.float32)
        )
    return out


def kernel(x, w, gamma, beta):
    if not _NC_CACHE:
        _NC_CACHE.append(_build_nc())
    nc = _NC_CACHE[0]
    maps = _prep_inputs(x, w, gamma, beta)
    res = run_bass_kernel_spmd(nc, maps, core_ids=list(range(N_CORES)))
    return _assemble(res.results)
